# revision 23
# baseline (speedup 1.0000x reference)
"""MultiHeadAttention (B=1, S=4096, E=1024, H=16) on 8 Trainium2 NeuronCores.

Sharding: tensor-parallel over heads. Core c computes heads 2c and 2c+1
(embed slice 128c:128c+128 of the QKV projections, column-parallel) plus its
row-slice of the output projection (row-parallel); the host sums the 8
partial fp16 outputs and adds the output bias.

Device kernel (identical SPMD program on every core, fp16 matmuls with fp32
PSUM accumulation):
  phase 1: Q^T/K^T/V^T = W_slice^T @ x^T (x^T pre-transposed on host),
           per-partition biases fused into the PSUM->SBUF copies on DVE;
           V^T transposed on the PE into V' = [V_h0 | 1 | V_h1 | 1] whose
           ones columns produce the softmax denominators inside the attn@V
           matmuls.
  phase 2: flash attention per 512-query chunk. The per-head contraction is
           only dh=64, so scores AND attn@V run as row-tiled (64x128 mode)
           concurrent matmul pairs: head0 on PE rows 0-63 (tile (0,0)),
           head1 on rows 64-127 (tile (64,0)), each pair finishing in the
           time of one matmul. attn@V accumulates per head into TWO psum
           banks (one per 64-key row group), summed during the epilogue.
           exp alternates between ScalarE (native, even k) and DVE (odd k,
           Schraudolph bit-trick: int16(A*s + B) bitcast to fp16 ~= exp(s/8),
           ~3% per-element, washes out through softmax normalization).
           Denominator broadcast on GpSimd + fast reciprocal on DVE.
           out_proj (row-parallel Wo slice) runs as a per-sq 128x128-mode
           burst in the next chunk's prologue, reusing the freed attn@V psum
           slots, evacuated by ScalarE into fp16 partials written to HBM.
"""
import numpy as np
from contextlib import ExitStack

EMBED = 1024
S = 4096
DCORE = 128          # embed cols per core (2 heads x 64)
DH = 64              # head dim
NCORES = 8
EC = 8               # e-chunks of 128 (contraction for projections)
NSC = 8              # s-chunks of 512 for projections
SQW = 512            # flash query-chunk width
NSQ = S // SQW       # 8
NK = 32              # key chunks of 128
VW = 2 * (DH + 1)    # 130: V' cols per s-tile  [V_h0 | 1 | V_h1 | 1]

# Schraudolph fp16 fast-exp constants: i16 = A*s + B, bitcast to f16 gives
# ~exp(0.125*s).  A = 0.125 * log2(e) * 1024; B centers the relative error.
A_EXP = 0.125 * 1024.0 * 1.4426950408889634
B_EXP = 15360.0 - 44.75

_CACHE = {}


def _build():
    import concourse.bacc as bacc
    import concourse.tile as tile
    from concourse import mybir

    F32 = mybir.dt.float32
    F16 = mybir.dt.float16
    I16 = mybir.dt.int16
    AF = mybir.ActivationFunctionType
    ALU = mybir.AluOpType

    nc = bacc.Bacc("TRN2", target_bir_lowering=False, debug=False)

    xT = nc.dram_tensor("xT", [EMBED, S], F16, kind="ExternalInput").ap()
    wq = nc.dram_tensor("wq", [EMBED, DCORE], F16, kind="ExternalInput").ap()
    wk = nc.dram_tensor("wk", [EMBED, DCORE], F16, kind="ExternalInput").ap()
    wv = nc.dram_tensor("wv", [EMBED, DCORE], F16, kind="ExternalInput").ap()
    wo = nc.dram_tensor("wo", [DCORE, EMBED], F16, kind="ExternalInput").ap()
    bq = nc.dram_tensor("bq", [DCORE, 1], F32, kind="ExternalInput").ap()
    bk = nc.dram_tensor("bk", [DCORE, 1], F32, kind="ExternalInput").ap()
    bv = nc.dram_tensor("bv", [DCORE, 1], F32, kind="ExternalInput").ap()
    ident = nc.dram_tensor("ident", [128, 128], F16, kind="ExternalInput").ap()
    ones = nc.dram_tensor("ones", [128, 2 * NK], F16, kind="ExternalInput").ap()
    out = nc.dram_tensor("out", [S, EMBED], F16, kind="ExternalOutput").ap()

    with tile.TileContext(nc) as tc, ExitStack() as ctx:
        cst = ctx.enter_context(tc.tile_pool(name="cst", bufs=1))
        big = ctx.enter_context(tc.tile_pool(name="big", bufs=1))

        # ---- constants / weights in SBUF ----
        wq_sb = cst.tile([128, EC * DCORE], F16, tag="wq")
        wk_sb = cst.tile([128, EC * DCORE], F16, tag="wk")
        wv_sb = cst.tile([128, EC * DCORE], F16, tag="wv")
        wo_sb = cst.tile([128, EMBED], F16, tag="wo")
        bq_sb = cst.tile([128, 1], F32, tag="bq")
        bk_sb = cst.tile([128, 1], F32, tag="bk")
        bv_sb = cst.tile([128, 1], F32, tag="bv")
        id_sb = cst.tile([128, 128], F16, tag="ident")

        # one DMA per weight: [e, d] -> [128, ec*d] e-chunk-major
        for w_dram, w_sb in ((wq, wq_sb), (wk, wk_sb), (wv, wv_sb)):
            nc.scalar.dma_start(
                w_sb[:].rearrange("p (ec n) -> p ec n", ec=EC),
                w_dram.rearrange("(ec p) n -> p ec n", p=128),
            )
        nc.scalar.dma_start(wo_sb[:], wo)
        nc.scalar.dma_start(bq_sb[:], bq)
        nc.scalar.dma_start(bk_sb[:], bk)
        nc.scalar.dma_start(bv_sb[:], bv)
        nc.scalar.dma_start(id_sb[:], ident)

        # ---- big SBUF tensors ----
        qT = big.tile([128, S], F16, tag="qT")   # rows 0:64 head0, 64:128 head1
        kT = big.tile([128, S], F16, tag="kT")
        vT = big.tile([128, S], F16, tag="vT")
        vp = big.tile([128, NK * VW], F16, tag="vp")   # V' per 128-row s-tile
        aT = big.tile([128, S], F16, tag="aT")   # normalized attn^T

        # ones columns of V': cols 64 and 129 of each 130-wide tile
        ones_sb = cst.tile([128, 2 * NK], F16, tag="ones_sb")
        nc.scalar.dma_start(ones_sb[:], ones)
        vp_r = vp[:].rearrange("p (t c) -> p t c", c=VW)
        nc.vector.tensor_copy(vp_r[:, :, DH::DH + 1],
                              ones_sb[:].rearrange("p (t c) -> p t c", c=2))
        # warm up the GpSimd broadcast library during phase 1 so the first
        # real partition_broadcast (mid-flash) doesn't pay the load
        gwarm = cst.tile([64, 8], F32, tag="gwarm")
        ones_f32 = cst.tile([1, 8], F32, tag="ones_f32")
        nc.vector.tensor_copy(ones_f32[:], ones_sb[0:1, 0:8])
        nc.gpsimd.partition_broadcast(gwarm[:], ones_f32[:])
        rwarm = cst.tile([64, 8], F32, tag="rwarm")
        nc.vector.reciprocal_approx_fast(rwarm[:], gwarm[:])

        # ---- phase 1: projections ----
        with (
            tc.tile_pool(name="xts", bufs=3) as xts_pool,
            tc.tile_pool(name="pps", bufs=2, space="PSUM") as pps,
            tc.tile_pool(name="vtp", bufs=2, space="PSUM") as vtps,
        ):
            for sc in range(NSC):
                xts = xts_pool.tile([128, EC * 512], F16, tag="xts")
                xts_r = xts[:].rearrange("p (ec n) -> p ec n", ec=EC)
                xT_r = xT[:, sc * 512:(sc + 1) * 512].rearrange(
                    "(ec p) n -> p ec n", p=128)
                for ec in range(EC):
                    nc.sync.dma_start(xts_r[:, ec:ec + 1], xT_r[:, ec:ec + 1])
                sl = slice(sc * 512, (sc + 1) * 512)
                psq = pps.tile([128, 512], F32, tag="psq")
                psk = pps.tile([128, 512], F32, tag="psk")
                psv = pps.tile([128, 512], F32, tag="psv")
                for ec in range(EC):
                    xsl = xts[:, ec * 512:(ec + 1) * 512]
                    st, sp = ec == 0, ec == EC - 1
                    nc.tensor.matmul(psq[:], wq_sb[:, ec * 128:(ec + 1) * 128], xsl,
                                     start=st, stop=sp)
                    nc.tensor.matmul(psk[:], wk_sb[:, ec * 128:(ec + 1) * 128], xsl,
                                     start=st, stop=sp)
                    nc.tensor.matmul(psv[:], wv_sb[:, ec * 128:(ec + 1) * 128], xsl,
                                     start=st, stop=sp)
                # PSUM -> SBUF with bias add (per-partition bias vectors)
                nc.vector.tensor_scalar_add(qT[:, sl], psq[:], bq_sb[:])
                nc.vector.tensor_scalar_add(kT[:, sl], psk[:], bk_sb[:])
                nc.vector.tensor_scalar_add(vT[:, sl], psv[:], bv_sb[:])
                # transpose V^T s-chunk into V' tiles (PE transpose);
                # evacuation split across ScalarE and DVE
                for t in range(4):
                    st_idx = 4 * sc + t
                    vtp = vtps.tile([128, 128], F16, tag="vtp")
                    nc.tensor.transpose(
                        vtp[:], vT[:, st_idx * 128:(st_idx + 1) * 128], id_sb[:])
                    base = st_idx * VW
                    nc.scalar.copy(vp[:, base:base + DH], vtp[:, 0:DH])
                    nc.vector.tensor_copy(
                        vp[:, base + DH + 1:base + 2 * DH + 1], vtp[:, DH:2 * DH])

        # ---- phase 2: flash attention + out_proj ----
        with (
            tc.tile_pool(name="scps", bufs=4, space="PSUM") as scps_pool,
            tc.tile_pool(name="avop", bufs=4, space="PSUM") as avop_pool,
            tc.tile_pool(name="ptp", bufs=12) as ptp,
            tc.tile_pool(name="eps", bufs=2) as eps,
            tc.tile_pool(name="osb", bufs=3) as osb_pool,
        ):
            def emit_scores_exp(sq, k):
                qsl = slice(sq * SQW, (sq + 1) * SQW)
                ksl = slice(k * 128, (k + 1) * 128)
                # row-tiled concurrent pair: h0 on PE rows 0-63, h1 on 64-127;
                # per-head psum tiles so the WAR chain clears per half.
                scA = scps_pool.tile([128, SQW], F32, tag="sc", name="scA")
                scB = scps_pool.tile([128, SQW], F32, tag="sc", name="scB")
                nc.tensor.matmul(scA[:], kT[0:64, ksl], qT[0:64, qsl],
                                 start=True, stop=True)
                nc.tensor.matmul(scB[:], kT[64:128, ksl], qT[64:128, qsl],
                                 start=True, stop=True)
                # exp, split over THREE engines: ScalarE native exp, DVE
                # Schraudolph direct from PSUM, GpSimd Schraudolph from an
                # f16 staging copy (ScalarE f32->f16 copies run 2x-accel).
                def emit_exp(sc_ps, eng):
                    pt = ptp.tile([128, SQW], F16, tag="pt", name="pt")
                    if eng == "S":
                        nc.scalar.activation(pt[:], sc_ps[:], AF.Exp, scale=0.125)
                    elif eng == "V":
                        nc.vector.tensor_scalar(pt[:].bitcast(I16), sc_ps[:],
                                                A_EXP, B_EXP, ALU.mult, ALU.add)
                    else:
                        pt16 = ptp.tile([128, SQW], F16, tag="pt16", name="pt16")
                        nc.scalar.copy(pt16[:], sc_ps[:])
                        nc.gpsimd.tensor_scalar(pt[:].bitcast(I16), pt16[:],
                                                A_EXP, B_EXP, ALU.mult, ALU.add)
                    return pt
                engA = "S"
                engB = "V"
                ptA = emit_exp(scA, engA)
                ptB = emit_exp(scB, engB)
                return ptA, ptB

            def emit_av(avs, k, pts):
                st, sp = k == 0, k == NK - 1
                vb = k * VW
                ptA, ptB = pts
                (av0A, av0B, av1A, av1B) = avs
                nc.tensor.matmul(av0A[0:65, :], vp[0:64, vb:vb + DH + 1],
                                 ptA[0:64, :], start=st, stop=sp)
                nc.tensor.matmul(av0B[0:65, :], vp[64:128, vb:vb + DH + 1],
                                 ptA[64:128, :], start=st, stop=sp)
                nc.tensor.matmul(av1A[0:65, :], vp[0:64, vb + DH + 1:vb + VW],
                                 ptB[0:64, :], start=st, stop=sp)
                nc.tensor.matmul(av1B[0:65, :], vp[64:128, vb + DH + 1:vb + VW],
                                 ptB[64:128, :], start=st, stop=sp)

            def emit_epilogue(sq, avs):
                (av0A, av0B, av1A, av1B) = avs
                qsl = slice(sq * SQW, (sq + 1) * SQW)
                for h, avA, avB in ((0, av0A, av0B), (1, av1A, av1B)):
                    # ScalarE evacuates one row-group (it sits next to PSUM),
                    # DVE adds the other: avoids a two-PSUM-operand DVE read.
                    avA_sb = eps.tile([65, SQW], F32, tag=f"avAsb{h}",
                                      name="avA_sb")
                    nc.scalar.copy(avA_sb[:], avA[0:65, :])
                    av_sb = eps.tile([65, SQW], F32, tag=f"avsb{h}",
                                     name="av_sb")
                    nc.vector.tensor_add(av_sb[:], avA_sb[:], avB[0:65, :])
                    den0 = eps.tile([1, SQW], F32, tag=f"den0{h}", name="den0")
                    nc.sync.dma_start(den0[:], av_sb[64:65, :])
                    dbc = eps.tile([64, SQW], F32, tag=f"dbc{h}", name="dbc")
                    nc.gpsimd.partition_broadcast(dbc[:], den0[:])
                    rbc = eps.tile([64, SQW], F32, tag=f"rbc{h}", name="rbc")
                    nc.vector.reciprocal_approx_fast(rbc[:], dbc[:])
                    if h == 0:
                        nc.vector.tensor_mul(aT[0:64, qsl], av_sb[0:64, :], rbc[:])
                    else:
                        a1 = eps.tile([64, SQW], F16, tag="a1", name="a1")
                        nc.vector.tensor_mul(a1[:], av_sb[0:64, :], rbc[:])
                        nc.sync.dma_start(aT[64:128, qsl], a1[:])

            def emit_outproj(sq, tiles):
                # 128x128-mode burst injected mid-k-loop (aT(sq) is long
                # ready there); psum tiles come from the scores pool (same
                # [128,512] slot), ScalarE evacuates, fp16 partial out.
                for t in tiles:
                    st_idx = sq * (SQW // 128) + t
                    asl = aT[:, st_idx * 128:(st_idx + 1) * 128]
                    osb = osb_pool.tile([128, EMBED], F16, tag="osb", name="osb")
                    for half in range(2):
                        op = scps_pool.tile([128, 512], F32, tag="sc", name="op")
                        nc.tensor.matmul(op[:], asl,
                                         wo_sb[:, half * 512:(half + 1) * 512],
                                         start=True, stop=True)
                        nc.scalar.copy(osb[:, half * 512:(half + 1) * 512], op[:])
                    nc.sync.dma_start(
                        out[st_idx * 128:(st_idx + 1) * 128, :], osb[:])

            def emit_outproj_tail(sq):
                # final out_proj: row-tiled pairs (64x128 mode, h0 tile can
                # start before the h1 aT DMA lands) combined on the
                # tail-idle ScalarE/DVE.
                for t in range(SQW // 128):
                    st_idx = sq * (SQW // 128) + t
                    asl = aT[:, st_idx * 128:(st_idx + 1) * 128]
                    osb = osb_pool.tile([128, EMBED], F16, tag="osb", name="osb")
                    for half in range(2):
                        hsl = slice(half * 512, (half + 1) * 512)
                        opA = scps_pool.tile([128, 512], F32, tag="sc", name="opA")
                        opB = scps_pool.tile([128, 512], F32, tag="sc", name="opB")
                        nc.tensor.matmul(opA[:], asl[0:64, :], wo_sb[0:64, hsl],
                                         start=True, stop=True)
                        nc.tensor.matmul(opB[:], asl[64:128, :], wo_sb[64:128, hsl],
                                         start=True, stop=True)
                        tmp = eps.tile([128, 512], F32, tag="optmp", name="tmp")
                        nc.scalar.copy(tmp[:], opA[:])
                        nc.vector.tensor_add(osb[:, hsl], tmp[:], opB[:])
                    nc.sync.dma_start(
                        out[st_idx * 128:(st_idx + 1) * 128, :], osb[:])

            prev_avs = None
            for sq in range(NSQ):
                # next-sq prologue FIRST so its exps precede the previous
                # epilogue's ops in the strict-FIFO Scalar/DVE queues
                pts = [emit_scores_exp(sq, k) for k in range(2)]
                if prev_avs is not None:
                    emit_epilogue(sq - 1, prev_avs)
                avs = [avop_pool.tile([128, SQW], F32, tag="av", name=f"av{i}")
                       for i in range(4)]
                for k in range(NK):
                    if sq >= 1 and k in (6, 14, 22, 30):
                        emit_outproj(sq - 1, ((k - 6) // 8,))
                    pt = pts.pop(0) if pts else emit_scores_exp(sq, k)
                    if k + 2 < NK:
                        pts.append(emit_scores_exp(sq, k + 2))
                    emit_av(avs, k, pt)
                prev_avs = avs
            emit_epilogue(NSQ - 1, prev_avs)
            emit_outproj_tail(NSQ - 1)

    nc.compile()
    return nc


def _get_nc():
    if "nc" not in _CACHE:
        _CACHE["nc"] = _build()
    return _CACHE["nc"]


def kernel(x, Wq, bq, Wk, bk, Wv, bv, Wo, bo):
    from concourse.bass_utils import run_bass_kernel_spmd

    x = np.asarray(x, dtype=np.float32)
    xT = np.ascontiguousarray(x.reshape(S, EMBED).T.astype(np.float16))
    eye = np.eye(128, dtype=np.float16)
    in_maps = []
    for c in range(NCORES):
        sl = slice(c * DCORE, (c + 1) * DCORE)
        in_maps.append({
            "xT": xT,
            "wq": np.ascontiguousarray(np.asarray(Wq, np.float32)[:, sl].astype(np.float16)),
            "wk": np.ascontiguousarray(np.asarray(Wk, np.float32)[:, sl].astype(np.float16)),
            "wv": np.ascontiguousarray(np.asarray(Wv, np.float32)[:, sl].astype(np.float16)),
            "wo": np.ascontiguousarray(np.asarray(Wo, np.float32)[sl, :].astype(np.float16)),
            "bq": np.asarray(bq, np.float32)[sl].reshape(DCORE, 1),
            "bk": np.asarray(bk, np.float32)[sl].reshape(DCORE, 1),
            "bv": np.asarray(bv, np.float32)[sl].reshape(DCORE, 1),
            "ident": eye,
            "ones": np.ones((128, 2 * NK), dtype=np.float16),
        })
    nc = _get_nc()
    res = run_bass_kernel_spmd(nc, in_maps, core_ids=list(range(NCORES)))
    acc = np.zeros((S, EMBED), dtype=np.float64)
    for c in range(NCORES):
        acc += res.results[c]["out"].astype(np.float64)
    acc += np.asarray(bo, np.float64)
    return acc.astype(np.float32).reshape(1, S, EMBED)
